# revision 14
# baseline (speedup 1.0000x reference)
"""AttentionPool Trainium2 kernel v2: fp8 DoubleRow matmuls, host-side
pre-transpose, bf16 staging.

Reference computation (per batch b of 32, S=2048, D=1024):
    xn = LayerNorm(x[b])                      # over D, eps 1e-5
    h = tanh(xn @ W1 + b1)
    scores = h @ W2 + b2                      # [S]
    w = softmax(scores)
    out[b] = sum_s w[s] * x[b, s, :]

Strategy: batch axis sharded over 8 cores (4 batches each). Host stages
x twice in bf16: [s, d] layout (LN stats + pooling values) and [d, s]
layout (pre-transposed, feeds matmul1) — no on-device transposes. Host
folds ln_gamma into W1 and ln_beta@W1+b1 into c2, and scales W1/W2 by 64
so fp8e4 (e4m3) quantization stays in the normal range; the inverse
scales ride the ACT activation `scale` operand.

Per core, per batch:
  - LN stats on DVE (bn_stats/bn_aggr + Newton rsqrt) in [s,d] layout;
    mu and rstd*16 bounce through DRAM and are broadcast-loaded as
    [128, S] tiles (per-free-column vectors for the transposed layout).
  - T-space normalize on DVE: xn8 = (xT - mu_b) * rs_b  -> fp8e4,
    written as [128, 2, S] d-pair tiles (DoubleRow operand layout).
  - matmul1: fp8 DoubleRow (K=256 per instruction), PSUM accumulate,
    tanh+c2 on ACT -> fp8 h pair tiles; scores via fp8 DoubleRow,
    exp on ACT (accum_out gives Z per chunk).
  - pooling via bf16 matmuls against the [s,d] x staging tiles kept in
    SBUF; divide by Z at the end.
Engine queues: GpSimd = x[s,d] loads; Sync = xT loads + stat stores +
output; Scalar(ACT) = broadcast loads + e-scatter bounces.
"""
import sys
import os

sys.path.insert(0, '/opt/trn_rl_repo')

import numpy as np

import concourse.bass as bass
import concourse.tile as tile
from concourse import bacc, mybir
from concourse.bass_utils import run_bass_kernel_spmd

P = 128
D = 1024
S = 2048
B = 32
NCORES = 8
BLOC = B // NCORES            # batches per core
ROWS = BLOC * S               # 8192 rows per core
DT = D // P                   # 8 d-tiles
ET = D // P                   # 8 e-tiles
DP = DT // 2                  # 4 d-pairs (DoubleRow)
EP = ET // 2                  # 4 e-pairs
SUBT = S // P                 # 16 subtiles per batch
NG = 4                        # subtiles per stats group
CHUNK = 512                   # matmul moving free dim
NCHUNK = S // CHUNK           # 4 chunks per batch

SW = 64.0                     # W1/W2 fp8 pre-scale (host)
SX = 16.0                     # xn fp8 pre-scale (device)
MM1_SCALE = 1.0 / (SW * SX)   # applied in tanh activation
SC_SCALE = 1.0 / SW           # applied in exp activation

f32 = mybir.dt.float32
bf16 = mybir.dt.bfloat16
fp8 = mybir.dt.float8e4
AF = mybir.ActivationFunctionType
ALU = mybir.AluOpType
DR = mybir.MatmulPerfMode.DoubleRow


def build_nc():
    nc = bacc.Bacc("TRN2", target_bir_lowering=False, num_devices=NCORES)

    xbf = nc.dram_tensor("xbf", [ROWS, D], bf16, kind="ExternalInput")
    xt = nc.dram_tensor("xt", [BLOC * D, S], bf16, kind="ExternalInput")
    w1q = nc.dram_tensor("w1q", [D, D], fp8, kind="ExternalInput")
    w2q = nc.dram_tensor("w2q", [D], fp8, kind="ExternalInput")
    c2v = nc.dram_tensor("c2v", [D], f32, kind="ExternalInput")
    b2s = nc.dram_tensor("b2s", [1, 1], f32, kind="ExternalInput")
    out = nc.dram_tensor("out", [BLOC, D], f32, kind="ExternalOutput")

    with tile.TileContext(nc) as tc:
        with (
            tc.tile_pool(name="consts", bufs=1) as consts,
            tc.tile_pool(name="xb", bufs=2) as xbp,            # [128,16,1024] bf16
            tc.tile_pool(name="stats", bufs=8) as statp,
            tc.tile_pool(name="bcast", bufs=8) as bcp,         # [128,512] bf16
            tc.tile_pool(name="xtp", bufs=4) as xtpp,          # [128,2,2048] bf16
            tc.tile_pool(name="xn8", bufs=8) as xn8p,          # [128,2,2048] fp8
            tc.tile_pool(name="h8", bufs=8) as h8p,            # [128,2,512] fp8
            tc.tile_pool(name="ec", bufs=4) as ecp,            # [1,512] bf16
            tc.tile_pool(name="epk", bufs=8) as epkp,          # [128,4] bf16
            tc.tile_pool(name="z", bufs=4) as zp,              # tiny scalars
            tc.tile_pool(name="ob", bufs=2) as obp,
            tc.tile_pool(name="psmm", bufs=5, space="PSUM") as psmm,
            tc.tile_pool(name="pssc", bufs=1, space="PSUM") as pssc,
            tc.tile_pool(name="pspool", bufs=2, space="PSUM") as pspool,
            tc.tile_pool(name="dram", bufs=8, space="DRAM") as dramp,
        ):
            # ---- constants ----
            w1_sb = consts.tile([P, DT, D], fp8)        # [d_in_tile, d_tile, e]
            nc.scalar.dma_start(w1_sb, w1q.ap().rearrange("(t p) e -> p t e", p=P))
            # dual-fp8 ldweights needs a 16B-aligned outer free step: pad
            # each e-tile's single weight column out to 16 bytes
            w2_sb = consts.tile([P, ET, 16], fp8)
            nc.scalar.dma_start(
                w2_sb[:, :, 0:1],
                w2q.ap().rearrange("(t p) -> p t", p=P).unsqueeze(2))
            c2_sb = consts.tile([P, ET], f32)
            nc.scalar.dma_start(c2_sb, c2v.ap().rearrange("(t p) -> p t", p=P))
            b2_sb = consts.tile([1, 1], f32)
            nc.sync.dma_start(b2_sb, b2s.ap())

            xbf3 = xbf.ap().rearrange("(b t p) d -> b t p d", b=BLOC, p=P)
            xt4 = xt.ap().rearrange("(b u p) s -> b p u s", b=BLOC, p=P)

            def phase1(b):
                """Load x[s,d] and xT, LN stats + Newton rsqrt + DRAM bounce
                per 512-token group so the first group's broadcast round-trip
                hides under the later groups' stats."""
                xb = xbp.tile([P, SUBT, D], bf16, tag="xb")
                xtps = []
                for i in range(DP):
                    xtp = xtpp.tile([P, 2, S], bf16, tag="xtp", name="xtp")
                    nc.sync.dma_start(xtp, xt4[b, :, 2 * i:2 * i + 2, :])
                    xtps.append(xtp)
                statd = dramp.tile([2, S], bf16, tag="statd", name="statd")
                bcasts = []
                for g in range(SUBT // NG):
                    t0 = g * NG
                    gs = slice(g * CHUNK, (g + 1) * CHUNK)
                    nc.gpsimd.dma_start(
                        xb[:, t0:t0 + NG, :],
                        xbf3[b, t0:t0 + NG].rearrange("t p d -> p t d"))
                    mvg = statp.tile([P, NG, 2], f32, tag="mvg")
                    for s in range(NG):
                        st = statp.tile([P, 2, 6], f32, tag="bnst")
                        nc.vector.bn_stats(st[:, 0, :], xb[:, t0 + s, 0:512])
                        nc.vector.bn_stats(st[:, 1, :], xb[:, t0 + s, 512:1024])
                        nc.vector.bn_aggr(mvg[:, s, :], st)
                    # rstd = rsqrt(var+eps): quake seed + 2 Newton steps
                    var = statp.tile([P, NG], f32, tag="var")
                    nc.vector.tensor_scalar(out=var, in0=mvg[:, :, 1],
                                            scalar1=1e-5, scalar2=0.5,
                                            op0=ALU.add, op1=ALU.mult)
                    y = statp.tile([P, NG], f32, tag="y")
                    yi = y.bitcast(mybir.dt.int32)
                    vi = var.bitcast(mybir.dt.int32)
                    nc.vector.tensor_scalar(out=yi, in0=vi, scalar1=0x800000,
                                            scalar2=None, op0=ALU.add)
                    nc.vector.tensor_scalar(out=yi, in0=yi, scalar1=1,
                                            scalar2=None,
                                            op0=ALU.logical_shift_right)
                    nc.vector.tensor_scalar(out=yi, in0=yi, scalar1=-1,
                                            scalar2=0x5f3759df,
                                            op0=ALU.mult, op1=ALU.add)
                    tny = statp.tile([P, NG], f32, tag="tny")
                    for _ in range(2):
                        nc.vector.tensor_tensor(tny, y, y, ALU.mult)
                        nc.vector.tensor_tensor(tny, tny, var, ALU.mult)
                        nc.vector.tensor_scalar(out=tny, in0=tny, scalar1=-1.0,
                                                scalar2=1.5,
                                                op0=ALU.mult, op1=ALU.add)
                        nc.vector.tensor_tensor(y, y, tny, ALU.mult)
                    # pack mu (bf16) and rstd*SX (bf16), bounce via DRAM
                    mub = statp.tile([P, NG], bf16, tag="mub")
                    nc.vector.tensor_copy(mub, mvg[:, :, 0])
                    rsb = statp.tile([P, NG], bf16, tag="rsb")
                    nc.vector.tensor_scalar(out=rsb, in0=y, scalar1=SX,
                                            scalar2=None, op0=ALU.mult)
                    nc.sync.dma_start(
                        statd[0:1, gs].rearrange("o (t p) -> p (o t)", p=P),
                        mub)
                    nc.sync.dma_start(
                        statd[1:2, gs].rearrange("o (t p) -> p (o t)", p=P),
                        rsb)
                    mu_bg = bcp.tile([P, CHUNK], bf16, tag="mu_b", name="mu_b")
                    nc.scalar.dma_start(
                        mu_bg, statd[0:1, gs].to_broadcast((P, CHUNK)))
                    rs_bg = bcp.tile([P, CHUNK], bf16, tag="rs_b", name="rs_b")
                    nc.scalar.dma_start(
                        rs_bg, statd[1:2, gs].to_broadcast((P, CHUNK)))
                    bcasts.append((mu_bg, rs_bg))
                return xb, xtps, bcasts

            def phase2(b, xtps, bcasts):
                """T-space normalize to fp8 pair tiles, chunk by chunk."""
                xn8s = [xn8p.tile([P, 2, S], fp8, tag="xn8", name="xn8")
                        for _ in range(DP)]
                for g in range(NCHUNK):
                    gs = slice(g * CHUNK, (g + 1) * CHUNK)
                    mu_bg, rs_bg = bcasts[g]
                    for i in range(DP):
                        for j in range(2):
                            nc.vector.tensor_tensor(xtps[i][:, j, gs],
                                                    xtps[i][:, j, gs], mu_bg,
                                                    ALU.subtract)
                            nc.vector.tensor_tensor(xn8s[i][:, j, gs],
                                                    xtps[i][:, j, gs], rs_bg,
                                                    ALU.mult)
                return xn8s

            def phase3(b, xn8s):
                """fp8 DoubleRow matmul1 + tanh + scores + exp per chunk."""
                zc = zp.tile([1, NCHUNK], f32, tag="zc", name="zc")
                epks = []
                for c in range(NCHUNK):
                    cs = slice(c * CHUNK, (c + 1) * CHUNK)
                    h8s = [h8p.tile([P, 2, CHUNK], fp8, tag="h8", name="h8")
                           for _ in range(EP)]
                    for e in range(ET):
                        ps = psmm.tile([P, CHUNK], f32, tag="psmm")
                        for i in range(DP):
                            nc.tensor.matmul(
                                ps, w1_sb[:, 2 * i:2 * i + 2,
                                          e * P:(e + 1) * P],
                                xn8s[i][:, :, cs],
                                start=(i == 0), stop=(i == DP - 1),
                                perf_mode=DR)
                        nc.scalar.activation(h8s[e // 2][:, e % 2, :], ps,
                                             AF.Tanh, bias=c2_sb[:, e:e + 1],
                                             scale=MM1_SCALE)
                    ps_sc = pssc.tile([1, CHUNK], f32, tag="pssc")
                    for k in range(EP):
                        nc.tensor.matmul(ps_sc,
                                         w2_sb[:, 2 * k:2 * k + 2, 0:1],
                                         h8s[k], start=(k == 0),
                                         stop=(k == EP - 1), perf_mode=DR)
                    ec = ecp.tile([1, CHUNK], bf16, tag="ec", name="ec")
                    nc.scalar.activation(ec, ps_sc, AF.Exp,
                                         bias=b2_sb[0:1, 0:1], scale=SC_SCALE,
                                         accum_out=zc[:, c:c + 1])
                    eb = dramp.tile([1, CHUNK], bf16, tag="eb", name="eb")
                    nc.scalar.dma_start(eb, ec)
                    epk = epkp.tile([P, NCHUNK], bf16, tag="epk", name="epk")
                    nc.scalar.dma_start(
                        epk, eb.rearrange("o (t p) -> (o p) t", p=P))
                    epks.append(epk)
                return zc, epks

            def phase4(b, xb, zc, epks):
                """Pooling matmuls vs SBUF-kept x[s,d], divide by Z, store."""
                pp0 = pspool.tile([1, CHUNK], f32, tag="pspool", name="pp0")
                pp1 = pspool.tile([1, CHUNK], f32, tag="pspool", name="pp1")
                for c in range(NCHUNK):
                    for t in range(NG):
                        tt = c * NG + t
                        nc.tensor.matmul(pp0, epks[c][:, t:t + 1],
                                         xb[:, tt, 0:512],
                                         start=(tt == 0), stop=(tt == SUBT - 1))
                        nc.tensor.matmul(pp1, epks[c][:, t:t + 1],
                                         xb[:, tt, 512:1024],
                                         start=(tt == 0), stop=(tt == SUBT - 1))
                zt = zp.tile([1, 1], f32, tag="zt")
                nc.vector.tensor_reduce(zt, zc,
                                        axis=mybir.AxisListType.X, op=ALU.add)
                rz = zp.tile([1, 1], f32, tag="rz")
                nc.vector.reciprocal(rz, zt)
                ob = obp.tile([1, D], f32, tag="ob")
                nc.scalar.activation(ob[:, 0:512], pp0, AF.Copy,
                                     scale=rz[0:1, 0:1])
                nc.scalar.activation(ob[:, 512:1024], pp1, AF.Copy,
                                     scale=rz[0:1, 0:1])
                nc.sync.dma_start(out.ap()[b:b + 1, :], ob)

            prev = None
            for b in range(BLOC):
                xb, xtps, bcasts = phase1(b)
                if prev is not None:
                    phase4(*prev)
                xn8s = phase2(b, xtps, bcasts)
                zc, epks = phase3(b, xn8s)
                prev = (b, xb, zc, epks)
            phase4(*prev)

    nc.compile()
    return nc


_NC_CACHE = {}


def _get_nc():
    if "nc" not in _NC_CACHE:
        _NC_CACHE["nc"] = build_nc()
    return _NC_CACHE["nc"]


def _prep_host(ln_gamma, ln_beta, W1, b1, W2, b2):
    import ml_dtypes
    f8 = ml_dtypes.float8_e4m3fn
    W1p = (np.asarray(ln_gamma, np.float32)[:, None]
           * np.asarray(W1, np.float32))
    w1q = np.clip(W1p * SW, -448, 448).astype(f8)
    c2 = (np.asarray(ln_beta, np.float32) @ np.asarray(W1, np.float32)
          + np.asarray(b1, np.float32))
    w2q = np.clip(
        np.ascontiguousarray(np.asarray(W2, np.float32)[:, 0]) * SW,
        -448, 448).astype(f8)
    b2s = np.asarray(b2, np.float32).reshape(1, 1)
    return np.ascontiguousarray(w1q), np.ascontiguousarray(c2), w2q, b2s


def run_cores(inputs, trace=False, **kw):
    import ml_dtypes
    x = np.asarray(inputs["x"], np.float32)
    w1q, c2, w2q, b2s = _prep_host(inputs["ln_gamma"], inputs["ln_beta"],
                                   inputs["W1"], inputs["b1"],
                                   inputs["W2"], inputs["b2"])
    xb16 = x.astype(ml_dtypes.bfloat16)          # [B, S, D]
    xt16 = np.ascontiguousarray(xb16.transpose(0, 2, 1))  # [B, D, S]
    nc = _get_nc()
    in_maps = []
    for c in range(NCORES):
        shard = np.ascontiguousarray(
            xb16[c * BLOC:(c + 1) * BLOC].reshape(ROWS, D))
        shardT = np.ascontiguousarray(
            xt16[c * BLOC:(c + 1) * BLOC].reshape(BLOC * D, S))
        in_maps.append(dict(xbf=shard, xt=shardT, w1q=w1q, w2q=w2q,
                            c2v=c2, b2s=b2s))
    res = run_bass_kernel_spmd(nc, in_maps, core_ids=list(range(NCORES)),
                               trace=trace, **kw)
    full = np.concatenate([res.results[c]["out"] for c in range(NCORES)], axis=0)
    return full, res


def kernel(**inputs) -> np.ndarray:
    out, _ = run_cores(inputs, trace=False)
    return out.astype(np.float32)


# revision 20
# speedup vs baseline: 2.1016x; 2.1016x over previous
"""AttentionPool Trainium2 kernel v2: fp8 DoubleRow matmuls, host-side
pre-transpose, bf16 staging.

Reference computation (per batch b of 32, S=2048, D=1024):
    xn = LayerNorm(x[b])                      # over D, eps 1e-5
    h = tanh(xn @ W1 + b1)
    scores = h @ W2 + b2                      # [S]
    w = softmax(scores)
    out[b] = sum_s w[s] * x[b, s, :]

Strategy: batch axis sharded over 8 cores (4 batches each). Host stages
x twice in bf16: [s, d] layout (LN stats + pooling values) and [d, s]
layout (pre-transposed, feeds matmul1) — no on-device transposes. Host
folds ln_gamma into W1 and ln_beta@W1+b1 into c2, and scales W1/W2 by 64
so fp8e4 (e4m3) quantization stays in the normal range; the inverse
scales ride the ACT activation `scale` operand.

Per core, per batch:
  - LN stats on DVE (bn_stats/bn_aggr + Newton rsqrt) in [s,d] layout;
    mu and rstd*16 bounce through DRAM and are broadcast-loaded as
    [128, S] tiles (per-free-column vectors for the transposed layout).
  - T-space normalize on DVE: xn8 = (xT - mu_b) * rs_b  -> fp8e4,
    written as [128, 2, S] d-pair tiles (DoubleRow operand layout).
  - matmul1: fp8 DoubleRow (K=256 per instruction), PSUM accumulate,
    tanh+c2 on ACT -> fp8 h pair tiles; scores via fp8 DoubleRow,
    exp on ACT (accum_out gives Z per chunk).
  - pooling via bf16 matmuls against the [s,d] x staging tiles kept in
    SBUF; divide by Z at the end.
Engine queues: GpSimd = x[s,d] loads; Sync = xT loads + stat stores +
output; Scalar(ACT) = broadcast loads + e-scatter bounces.
"""
import sys
import os

sys.path.insert(0, '/opt/trn_rl_repo')

import numpy as np

import concourse.bass as bass
import concourse.tile as tile
from concourse import bacc, mybir
from concourse.bass_utils import run_bass_kernel_spmd

P = 128
D = 1024
S = 2048
B = 32
NCORES = 8
BLOC = B // NCORES            # batches per core
ROWS = BLOC * S               # 8192 rows per core
DT = D // P                   # 8 d-tiles
ET = D // P                   # 8 e-tiles
DP = DT // 2                  # 4 d-pairs (DoubleRow)
EP = ET // 2                  # 4 e-pairs
SUBT = S // P                 # 16 subtiles per batch
NG = 4                        # subtiles per stats group
CHUNK = 512                   # matmul moving free dim
NCHUNK = S // CHUNK           # 4 chunks per batch

SW = 64.0                     # W1/W2 fp8 pre-scale (host)
SX = 16.0                     # xn fp8 pre-scale (device)
MM1_SCALE = 1.0 / (SW * SX)   # applied in tanh activation
SC_SCALE = 1.0 / SW           # applied in exp activation

f32 = mybir.dt.float32
bf16 = mybir.dt.bfloat16
fp8 = mybir.dt.float8e4
AF = mybir.ActivationFunctionType
ALU = mybir.AluOpType
DR = mybir.MatmulPerfMode.DoubleRow


def build_nc():
    nc = bacc.Bacc("TRN2", target_bir_lowering=False, num_devices=NCORES)

    xbf = nc.dram_tensor("xbf", [ROWS, D], bf16, kind="ExternalInput")
    xt = nc.dram_tensor("xt", [BLOC * D, S], bf16, kind="ExternalInput")
    w1q = nc.dram_tensor("w1q", [D, D], fp8, kind="ExternalInput")
    w2q = nc.dram_tensor("w2q", [D], fp8, kind="ExternalInput")
    c2v = nc.dram_tensor("c2v", [D], f32, kind="ExternalInput")
    b2s = nc.dram_tensor("b2s", [1, 1], f32, kind="ExternalInput")
    eye = nc.dram_tensor("eye", [P, P], bf16, kind="ExternalInput")
    out = nc.dram_tensor("out", [BLOC, D], f32, kind="ExternalOutput")

    with tile.TileContext(nc) as tc:
        with (
            tc.tile_pool(name="consts", bufs=1) as consts,
            tc.tile_pool(name="xb", bufs=2) as xbp,            # [128,16,1024] bf16
            tc.tile_pool(name="stats", bufs=8) as statp,
            tc.tile_pool(name="bcast", bufs=8) as bcp,         # [128,512] bf16
            tc.tile_pool(name="xtp", bufs=4) as xtpp,          # [128,2,2048] bf16
            tc.tile_pool(name="xn8", bufs=8) as xn8p,          # [128,2,2048] fp8
            tc.tile_pool(name="h8", bufs=8) as h8p,            # [128,2,512] fp8
            tc.tile_pool(name="ec", bufs=4) as ecp,            # [1,512] bf16
            tc.tile_pool(name="epk", bufs=8) as epkp,          # [128,4] bf16
            tc.tile_pool(name="z", bufs=4) as zp,              # tiny scalars
            tc.tile_pool(name="ob", bufs=2) as obp,
            tc.tile_pool(name="psmm", bufs=4, space="PSUM") as psmm,
            tc.tile_pool(name="pssc", bufs=1, space="PSUM") as pssc,
            tc.tile_pool(name="pspool", bufs=2, space="PSUM") as pspool,
            tc.tile_pool(name="pst", bufs=1, space="PSUM") as pstp,
            tc.tile_pool(name="dram", bufs=8, space="DRAM") as dramp,
        ):
            # ---- constants ----
            w1_sb = consts.tile([P, DT, D], fp8)        # [d_in_tile, d_tile, e]
            nc.scalar.dma_start(w1_sb, w1q.ap().rearrange("(t p) e -> p t e", p=P))
            # dual-fp8 ldweights needs a 16B-aligned outer free step: pad
            # each e-tile's single weight column out to 16 bytes
            w2_sb = consts.tile([P, ET, 16], fp8)
            nc.scalar.dma_start(
                w2_sb[:, :, 0:1],
                w2q.ap().rearrange("(t p) -> p t", p=P).unsqueeze(2))
            c2_sb = consts.tile([P, ET], f32)
            nc.scalar.dma_start(c2_sb, c2v.ap().rearrange("(t p) -> p t", p=P))
            b2_sb = consts.tile([1, 1], f32)
            nc.sync.dma_start(b2_sb, b2s.ap())
            eye_sb = consts.tile([P, P], bf16)
            nc.sync.dma_start(eye_sb, eye.ap())

            xbf3 = xbf.ap().rearrange("(b t p) d -> b t p d", b=BLOC, p=P)
            xt4 = xt.ap().rearrange("(b u p) s -> b p u s", b=BLOC, p=P)

            def phase1(b):
                """Load x[s,d] and xT, LN stats + Newton rsqrt + DRAM bounce
                per 512-token group so the first group's broadcast round-trip
                hides under the later groups' stats."""
                xb = xbp.tile([P, SUBT, D], bf16, tag="xb")
                xtps = []
                for i in range(DP):
                    xtp = xtpp.tile([P, 2, S], bf16, tag="xtp", name="xtp")
                    nc.sync.dma_start(xtp, xt4[b, :, 2 * i:2 * i + 2, :])
                    xtps.append(xtp)
                statd = dramp.tile([2, S], bf16, tag="statd", name="statd")
                bcasts = []
                for g in range(SUBT // NG):
                    t0 = g * NG
                    gs = slice(g * CHUNK, (g + 1) * CHUNK)
                    nc.gpsimd.dma_start(
                        xb[:, t0:t0 + NG, :],
                        xbf3[b, t0:t0 + NG].rearrange("t p d -> p t d"))
                    mvg = statp.tile([P, NG, 2], f32, tag="mvg")
                    for s in range(NG):
                        st = statp.tile([P, 2, 6], f32, tag="bnst")
                        nc.vector.bn_stats(st[:, 0, :], xb[:, t0 + s, 0:512])
                        nc.vector.bn_stats(st[:, 1, :], xb[:, t0 + s, 512:1024])
                        nc.vector.bn_aggr(mvg[:, s, :], st)
                    # rstd = rsqrt(var+eps): quake seed + 2 Newton steps
                    var = statp.tile([P, NG], f32, tag="var")
                    nc.vector.tensor_scalar(out=var, in0=mvg[:, :, 1],
                                            scalar1=1e-5, scalar2=0.5,
                                            op0=ALU.add, op1=ALU.mult)
                    y = statp.tile([P, NG], f32, tag="y")
                    yi = y.bitcast(mybir.dt.int32)
                    vi = var.bitcast(mybir.dt.int32)
                    nc.vector.tensor_scalar(out=yi, in0=vi, scalar1=0x800000,
                                            scalar2=None, op0=ALU.add)
                    nc.vector.tensor_scalar(out=yi, in0=yi, scalar1=1,
                                            scalar2=None,
                                            op0=ALU.logical_shift_right)
                    nc.vector.tensor_scalar(out=yi, in0=yi, scalar1=-1,
                                            scalar2=0x5f3759df,
                                            op0=ALU.mult, op1=ALU.add)
                    tny = statp.tile([P, NG], f32, tag="tny")
                    for _ in range(2):
                        nc.vector.tensor_tensor(tny, y, y, ALU.mult)
                        nc.vector.tensor_tensor(tny, tny, var, ALU.mult)
                        nc.vector.tensor_scalar(out=tny, in0=tny, scalar1=-1.0,
                                                scalar2=1.5,
                                                op0=ALU.mult, op1=ALU.add)
                        nc.vector.tensor_tensor(y, y, tny, ALU.mult)
                    # pack mu (bf16) and rstd*SX (bf16) side by side, PE
                    # transpose [128,8]->[8,128] so the DRAM bounce store is
                    # contiguous (8 descriptors instead of a 512-way scatter)
                    mr = statp.tile([P, 2 * NG], bf16, tag="mr")
                    nc.vector.tensor_copy(mr[:, 0:NG], mvg[:, :, 0])
                    nc.vector.tensor_scalar(out=mr[:, NG:2 * NG], in0=y,
                                            scalar1=SX, scalar2=None,
                                            op0=ALU.mult)
                    mrt = pstp.tile([2 * NG, P], bf16, tag="mrt")
                    nc.tensor.transpose(mrt, mr, eye_sb)
                    mrs = statp.tile([2 * NG, P], bf16, tag="mrs")
                    nc.scalar.activation(mrs, mrt, AF.Copy)
                    nc.sync.dma_start(statd[0:2, gs], mrs)
                    mu_bg = bcp.tile([P, CHUNK], bf16, tag="mu_b", name="mu_b")
                    nc.scalar.dma_start(
                        mu_bg, statd[0:1, gs].to_broadcast((P, CHUNK)))
                    rs_bg = bcp.tile([P, CHUNK], bf16, tag="rs_b", name="rs_b")
                    nc.scalar.dma_start(
                        rs_bg, statd[1:2, gs].to_broadcast((P, CHUNK)))
                    bcasts.append((mu_bg, rs_bg))
                return xb, xtps, bcasts

            def phase2(b, xtps, bcasts):
                """T-space normalize to fp8 pair tiles, chunk by chunk."""
                xn8s = [xn8p.tile([P, 2, S], fp8, tag="xn8", name="xn8")
                        for _ in range(DP)]
                for g in range(NCHUNK):
                    gs = slice(g * CHUNK, (g + 1) * CHUNK)
                    mu_bg, rs_bg = bcasts[g]
                    for i in range(DP):
                        for j in range(2):
                            nc.vector.tensor_tensor(xtps[i][:, j, gs],
                                                    xtps[i][:, j, gs], mu_bg,
                                                    ALU.subtract)
                            nc.vector.tensor_tensor(xn8s[i][:, j, gs],
                                                    xtps[i][:, j, gs], rs_bg,
                                                    ALU.mult)
                return xn8s

            def phase3(b, xn8s):
                """fp8 DoubleRow matmul1 + tanh + scores + exp per chunk."""
                zc = zp.tile([1, NCHUNK], f32, tag="zc", name="zc")
                epks = []
                for c in range(NCHUNK):
                    cs = slice(c * CHUNK, (c + 1) * CHUNK)
                    h8s = [h8p.tile([P, 2, CHUNK], fp8, tag="h8", name="h8")
                           for _ in range(EP)]
                    for e in range(ET):
                        ps = psmm.tile([P, CHUNK], f32, tag="psmm")
                        for i in range(DP):
                            nc.tensor.matmul(
                                ps, w1_sb[:, 2 * i:2 * i + 2,
                                          e * P:(e + 1) * P],
                                xn8s[i][:, :, cs],
                                start=(i == 0), stop=(i == DP - 1),
                                perf_mode=DR)
                        nc.scalar.activation(h8s[e // 2][:, e % 2, :], ps,
                                             AF.Tanh, bias=c2_sb[:, e:e + 1],
                                             scale=MM1_SCALE)
                    ps_sc = pssc.tile([1, CHUNK], f32, tag="pssc")
                    for k in range(EP):
                        nc.tensor.matmul(ps_sc,
                                         w2_sb[:, 2 * k:2 * k + 2, 0:1],
                                         h8s[k], start=(k == 0),
                                         stop=(k == EP - 1), perf_mode=DR)
                    ec = ecp.tile([1, CHUNK], bf16, tag="ec", name="ec")
                    nc.scalar.activation(ec, ps_sc, AF.Exp,
                                         bias=b2_sb[0:1, 0:1], scale=SC_SCALE,
                                         accum_out=zc[:, c:c + 1])
                    eb = dramp.tile([1, CHUNK], bf16, tag="eb", name="eb")
                    nc.scalar.dma_start(eb, ec)
                    epk = epkp.tile([P, NCHUNK], bf16, tag="epk", name="epk")
                    nc.scalar.dma_start(
                        epk, eb.rearrange("o (t p) -> (o p) t", p=P))
                    epks.append(epk)
                return zc, epks

            def phase4(b, xb, zc, epks):
                """Pooling matmuls vs SBUF-kept x[s,d], divide by Z, store."""
                pp0 = pspool.tile([1, CHUNK], f32, tag="pspool", name="pp0")
                pp1 = pspool.tile([1, CHUNK], f32, tag="pspool", name="pp1")
                for c in range(NCHUNK):
                    for t in range(NG):
                        tt = c * NG + t
                        nc.tensor.matmul(pp0, epks[c][:, t:t + 1],
                                         xb[:, tt, 0:512],
                                         start=(tt == 0), stop=(tt == SUBT - 1))
                        nc.tensor.matmul(pp1, epks[c][:, t:t + 1],
                                         xb[:, tt, 512:1024],
                                         start=(tt == 0), stop=(tt == SUBT - 1))
                zt = zp.tile([1, 1], f32, tag="zt")
                nc.vector.tensor_reduce(zt, zc,
                                        axis=mybir.AxisListType.X, op=ALU.add)
                rz = zp.tile([1, 1], f32, tag="rz")
                nc.vector.reciprocal(rz, zt)
                ob = obp.tile([1, D], f32, tag="ob")
                nc.scalar.activation(ob[:, 0:512], pp0, AF.Copy,
                                     scale=rz[0:1, 0:1])
                nc.scalar.activation(ob[:, 512:1024], pp1, AF.Copy,
                                     scale=rz[0:1, 0:1])
                nc.sync.dma_start(out.ap()[b:b + 1, :], ob)

            prev = None
            for b in range(BLOC):
                xb, xtps, bcasts = phase1(b)
                if prev is not None:
                    phase4(*prev)
                xn8s = phase2(b, xtps, bcasts)
                zc, epks = phase3(b, xn8s)
                prev = (b, xb, zc, epks)
            phase4(*prev)

    nc.compile()
    return nc


_NC_CACHE = {}


def _get_nc():
    if "nc" not in _NC_CACHE:
        _NC_CACHE["nc"] = build_nc()
    return _NC_CACHE["nc"]


def _prep_host(ln_gamma, ln_beta, W1, b1, W2, b2):
    import ml_dtypes
    f8 = ml_dtypes.float8_e4m3fn
    W1p = (np.asarray(ln_gamma, np.float32)[:, None]
           * np.asarray(W1, np.float32))
    w1q = np.clip(W1p * SW, -448, 448).astype(f8)
    c2 = (np.asarray(ln_beta, np.float32) @ np.asarray(W1, np.float32)
          + np.asarray(b1, np.float32))
    w2q = np.clip(
        np.ascontiguousarray(np.asarray(W2, np.float32)[:, 0]) * SW,
        -448, 448).astype(f8)
    b2s = np.asarray(b2, np.float32).reshape(1, 1)
    return np.ascontiguousarray(w1q), np.ascontiguousarray(c2), w2q, b2s


def run_cores(inputs, trace=False, **kw):
    import ml_dtypes
    x = np.asarray(inputs["x"], np.float32)
    w1q, c2, w2q, b2s = _prep_host(inputs["ln_gamma"], inputs["ln_beta"],
                                   inputs["W1"], inputs["b1"],
                                   inputs["W2"], inputs["b2"])
    xb16 = x.astype(ml_dtypes.bfloat16)          # [B, S, D]
    xt16 = np.ascontiguousarray(xb16.transpose(0, 2, 1))  # [B, D, S]
    nc = _get_nc()
    in_maps = []
    for c in range(NCORES):
        shard = np.ascontiguousarray(
            xb16[c * BLOC:(c + 1) * BLOC].reshape(ROWS, D))
        shardT = np.ascontiguousarray(
            xt16[c * BLOC:(c + 1) * BLOC].reshape(BLOC * D, S))
        in_maps.append(dict(xbf=shard, xt=shardT, w1q=w1q, w2q=w2q,
                            c2v=c2, b2s=b2s,
                            eye=np.eye(P, dtype=ml_dtypes.bfloat16)))
    res = run_bass_kernel_spmd(nc, in_maps, core_ids=list(range(NCORES)),
                               trace=trace, **kw)
    full = np.concatenate([res.results[c]["out"] for c in range(NCORES)], axis=0)
    return full, res


def kernel(**inputs) -> np.ndarray:
    out, _ = run_cores(inputs, trace=False)
    return out.astype(np.float32)


# revision 26
# speedup vs baseline: 2.2126x; 1.0528x over previous
"""AttentionPool Trainium2 kernel v2: fp8 DoubleRow matmuls, host-side
pre-transpose, bf16 staging.

Reference computation (per batch b of 32, S=2048, D=1024):
    xn = LayerNorm(x[b])                      # over D, eps 1e-5
    h = tanh(xn @ W1 + b1)
    scores = h @ W2 + b2                      # [S]
    w = softmax(scores)
    out[b] = sum_s w[s] * x[b, s, :]

Strategy: batch axis sharded over 8 cores (4 batches each). Host stages
x twice in bf16: [s, d] layout (LN stats + pooling values) and [d, s]
layout (pre-transposed, feeds matmul1) — no on-device transposes. Host
folds ln_gamma into W1 and ln_beta@W1+b1 into c2, and scales W1/W2 by 64
so fp8e4 (e4m3) quantization stays in the normal range; the inverse
scales ride the ACT activation `scale` operand.

Per core, per batch:
  - LN stats on DVE (bn_stats/bn_aggr + Newton rsqrt) in [s,d] layout;
    mu and rstd*16 bounce through DRAM and are broadcast-loaded as
    [128, S] tiles (per-free-column vectors for the transposed layout).
  - T-space normalize on DVE: xn8 = (xT - mu_b) * rs_b  -> fp8e4,
    written as [128, 2, S] d-pair tiles (DoubleRow operand layout).
  - matmul1: fp8 DoubleRow (K=256 per instruction), PSUM accumulate,
    tanh+c2 on ACT -> fp8 h pair tiles; scores via fp8 DoubleRow,
    exp on ACT (accum_out gives Z per chunk).
  - pooling via bf16 matmuls against the [s,d] x staging tiles kept in
    SBUF; divide by Z at the end.
Engine queues: GpSimd = x[s,d] loads; Sync = xT loads + stat stores +
output; Scalar(ACT) = broadcast loads + e-scatter bounces.
"""
import sys
import os

sys.path.insert(0, '/opt/trn_rl_repo')

import numpy as np

import concourse.bass as bass
import concourse.tile as tile
from concourse import bacc, mybir
from concourse.bass_utils import run_bass_kernel_spmd

P = 128
D = 1024
S = 2048
B = 32
NCORES = 8
BLOC = B // NCORES            # batches per core
ROWS = BLOC * S               # 8192 rows per core
DT = D // P                   # 8 d-tiles
ET = D // P                   # 8 e-tiles
DP = DT // 2                  # 4 d-pairs (DoubleRow)
EP = ET // 2                  # 4 e-pairs
SUBT = S // P                 # 16 subtiles per batch
NG = 4                        # subtiles per stats group
CHUNK = 512                   # matmul moving free dim
NCHUNK = S // CHUNK           # 4 chunks per batch

SW = 64.0                     # W1/W2 fp8 pre-scale (host)
SX = 16.0                     # xn fp8 pre-scale (device)
MM1_SCALE = 1.0 / (SW * SX)   # applied in tanh activation
SC_SCALE = 1.0 / SW           # applied in exp activation

f32 = mybir.dt.float32
bf16 = mybir.dt.bfloat16
fp8 = mybir.dt.float8e4
AF = mybir.ActivationFunctionType
ALU = mybir.AluOpType
DR = mybir.MatmulPerfMode.DoubleRow
DRSWI = mybir.MatmulPerfMode.DoubleRowSwInterleave


def build_nc():
    nc = bacc.Bacc("TRN2", target_bir_lowering=False, num_devices=NCORES)

    xbf = nc.dram_tensor("xbf", [ROWS, D], bf16, kind="ExternalInput")
    xt = nc.dram_tensor("xt", [BLOC * D, S], bf16, kind="ExternalInput")
    # W1 pre-interleaved on host for DoubleRowSwInterleave ldweights:
    # per partition d_p, free dim = [A_127 B_127 ... A_0 B_0] per (pair, e)
    w1q = nc.dram_tensor("w1q", [P, DP * ET * 2 * P], fp8,
                         kind="ExternalInput")
    w2q = nc.dram_tensor("w2q", [D], fp8, kind="ExternalInput")
    c2v = nc.dram_tensor("c2v", [D], f32, kind="ExternalInput")
    b2s = nc.dram_tensor("b2s", [1, 1], f32, kind="ExternalInput")
    eye = nc.dram_tensor("eye", [P, P], bf16, kind="ExternalInput")
    out = nc.dram_tensor("out", [BLOC, D], f32, kind="ExternalOutput")

    with tile.TileContext(nc) as tc:
        with (
            tc.tile_pool(name="consts", bufs=1) as consts,
            tc.tile_pool(name="xb", bufs=2) as xbp,            # [128,16,1024] bf16
            tc.tile_pool(name="stats", bufs=8) as statp,
            tc.tile_pool(name="bcast", bufs=8) as bcp,         # [128,512] bf16
            tc.tile_pool(name="xtp", bufs=4) as xtpp,          # [128,2,2048] bf16
            tc.tile_pool(name="xn8", bufs=8) as xn8p,          # [128,2,2048] fp8
            tc.tile_pool(name="h8", bufs=8) as h8p,            # [128,2,512] fp8
            tc.tile_pool(name="ec", bufs=4) as ecp,            # [1,512] bf16
            tc.tile_pool(name="epk", bufs=8) as epkp,          # [128,4] bf16
            tc.tile_pool(name="z", bufs=4) as zp,              # tiny scalars
            tc.tile_pool(name="ob", bufs=2) as obp,
            tc.tile_pool(name="psmm", bufs=4, space="PSUM") as psmm,
            tc.tile_pool(name="pssc", bufs=1, space="PSUM") as pssc,
            tc.tile_pool(name="pspool", bufs=2, space="PSUM") as pspool,
            tc.tile_pool(name="pst", bufs=1, space="PSUM") as pstp,
            tc.tile_pool(name="dram", bufs=8, space="DRAM") as dramp,
        ):
            # ---- constants ----
            w1_sb = consts.tile([P, DP, ET, 2 * P], fp8)   # interleaved pairs
            nc.scalar.dma_start(
                w1_sb, w1q.ap().rearrange("p (i e m) -> p i e m", i=DP, e=ET))
            # dual-fp8 ldweights needs a 16B-aligned outer free step: pad
            # each e-tile's single weight column out to 16 bytes
            w2_sb = consts.tile([P, ET, 16], fp8)
            nc.scalar.dma_start(
                w2_sb[:, :, 0:1],
                w2q.ap().rearrange("(t p) -> p t", p=P).unsqueeze(2))
            c2_sb = consts.tile([P, ET], f32)
            nc.scalar.dma_start(c2_sb, c2v.ap().rearrange("(t p) -> p t", p=P))
            b2_sb = consts.tile([1, 1], f32)
            nc.sync.dma_start(b2_sb, b2s.ap())
            eye_sb = consts.tile([P, P], bf16)
            nc.sync.dma_start(eye_sb, eye.ap())

            xbf3 = xbf.ap().rearrange("(b t p) d -> b t p d", b=BLOC, p=P)
            xt4 = xt.ap().rearrange("(b u p) s -> b p u s", b=BLOC, p=P)

            HB = S // 2           # half-batch token granularity
            HSUB = SUBT // 2      # 8 subtiles per half

            def phase1(b):
                """Load x[s,d] and xT; LN stats + quake+1-Newton rsqrt + DRAM
                bounce per half-batch (1024 tokens) so the first half's
                broadcast round-trip hides under the second half's stats."""
                xb = xbp.tile([P, SUBT, D], bf16, tag="xb")
                xtps = []
                for i in range(DP):
                    xtp = xtpp.tile([P, 2, S], bf16, tag="xtp", name="xtp")
                    nc.sync.dma_start(xtp, xt4[b, :, 2 * i:2 * i + 2, :])
                    xtps.append(xtp)
                statd = dramp.tile([2, S], bf16, tag="statd", name="statd")
                bcasts = []
                for h in range(2):
                    t0 = h * HSUB
                    hs = slice(h * HB, (h + 1) * HB)
                    mvh = statp.tile([P, HSUB, 2], f32, tag="mvh")
                    for g in range(2):
                        tg = t0 + g * NG
                        nc.gpsimd.dma_start(
                            xb[:, tg:tg + NG, :],
                            xbf3[b, tg:tg + NG].rearrange("t p d -> p t d"))
                        for s in range(NG):
                            st = statp.tile([P, 2, 6], f32, tag="bnst")
                            nc.vector.bn_stats(st[:, 0, :],
                                               xb[:, tg + s, 0:512])
                            nc.vector.bn_stats(st[:, 1, :],
                                               xb[:, tg + s, 512:1024])
                            nc.vector.bn_aggr(mvh[:, g * NG + s, :], st)
                    # rstd = rsqrt(var+eps): quake seed + 1 Newton step
                    var = statp.tile([P, HSUB], f32, tag="var")
                    nc.vector.tensor_scalar(out=var, in0=mvh[:, :, 1],
                                            scalar1=1e-5, scalar2=0.5,
                                            op0=ALU.add, op1=ALU.mult)
                    y = statp.tile([P, HSUB], f32, tag="y")
                    yi = y.bitcast(mybir.dt.int32)
                    vi = var.bitcast(mybir.dt.int32)
                    nc.vector.tensor_scalar(out=yi, in0=vi, scalar1=0x800000,
                                            scalar2=None, op0=ALU.add)
                    nc.vector.tensor_scalar(out=yi, in0=yi, scalar1=1,
                                            scalar2=None,
                                            op0=ALU.logical_shift_right)
                    nc.vector.tensor_scalar(out=yi, in0=yi, scalar1=-1,
                                            scalar2=0x5f3759df,
                                            op0=ALU.mult, op1=ALU.add)
                    tny = statp.tile([P, HSUB], f32, tag="tny")
                    nc.vector.tensor_tensor(tny, y, y, ALU.mult)
                    nc.vector.tensor_tensor(tny, tny, var, ALU.mult)
                    nc.vector.tensor_scalar(out=tny, in0=tny, scalar1=-1.0,
                                            scalar2=1.5,
                                            op0=ALU.mult, op1=ALU.add)
                    nc.vector.tensor_tensor(y, y, tny, ALU.mult)
                    # pack mu | rstd*SX (bf16), PE transpose [128,16]->[16,128]
                    # so the DRAM bounce store is contiguous
                    mr = statp.tile([P, 2 * HSUB], bf16, tag="mr")
                    nc.vector.tensor_copy(mr[:, 0:HSUB], mvh[:, :, 0])
                    nc.vector.tensor_scalar(out=mr[:, HSUB:2 * HSUB], in0=y,
                                            scalar1=SX, scalar2=None,
                                            op0=ALU.mult)
                    mrt = pstp.tile([2 * HSUB, P], bf16, tag="mrt")
                    nc.tensor.transpose(mrt, mr, eye_sb)
                    mrs = statp.tile([2 * HSUB, P], bf16, tag="mrs")
                    nc.scalar.activation(mrs, mrt, AF.Copy)
                    nc.sync.dma_start(statd[0:2, hs], mrs)
                    mu_bh = bcp.tile([P, HB], bf16, tag="mu_b", name="mu_b")
                    nc.scalar.dma_start(
                        mu_bh, statd[0:1, hs].to_broadcast((P, HB)))
                    rs_bh = bcp.tile([P, HB], bf16, tag="rs_b", name="rs_b")
                    nc.scalar.dma_start(
                        rs_bh, statd[1:2, hs].to_broadcast((P, HB)))
                    bcasts.append((mu_bh, rs_bh))
                return xb, xtps, bcasts

            def phase2(b, xtps, bcasts):
                """T-space normalize to fp8 pair tiles, half by half."""
                xn8s = [xn8p.tile([P, 2, S], fp8, tag="xn8", name="xn8")
                        for _ in range(DP)]
                for h in range(2):
                    hs = slice(h * HB, (h + 1) * HB)
                    mu_bh, rs_bh = bcasts[h]
                    for i in range(DP):
                        for j in range(2):
                            nc.vector.tensor_tensor(xtps[i][:, j, hs],
                                                    xtps[i][:, j, hs], mu_bh,
                                                    ALU.subtract)
                            nc.vector.tensor_tensor(xn8s[i][:, j, hs],
                                                    xtps[i][:, j, hs], rs_bh,
                                                    ALU.mult)
                return xn8s

            def phase3(b, xn8s):
                """fp8 DoubleRow matmul1 + tanh + scores + exp per chunk."""
                zc = zp.tile([1, NCHUNK], f32, tag="zc", name="zc")
                epks = []
                for c in range(NCHUNK):
                    cs = slice(c * CHUNK, (c + 1) * CHUNK)
                    h8s = [h8p.tile([P, 2, CHUNK], fp8, tag="h8", name="h8")
                           for _ in range(EP)]
                    for e in range(ET):
                        ps = psmm.tile([P, CHUNK], f32, tag="psmm")
                        for i in range(DP):
                            nc.tensor.matmul(
                                ps, w1_sb[:, i, e, :],
                                xn8s[i][:, :, cs],
                                start=(i == 0), stop=(i == DP - 1),
                                perf_mode=DRSWI)
                        nc.scalar.activation(h8s[e // 2][:, e % 2, :], ps,
                                             AF.Tanh, bias=c2_sb[:, e:e + 1],
                                             scale=MM1_SCALE)
                    ps_sc = pssc.tile([1, CHUNK], f32, tag="pssc")
                    for k in range(EP):
                        nc.tensor.matmul(ps_sc,
                                         w2_sb[:, 2 * k:2 * k + 2, 0:1],
                                         h8s[k], start=(k == 0),
                                         stop=(k == EP - 1), perf_mode=DR)
                    ec = ecp.tile([1, CHUNK], bf16, tag="ec", name="ec")
                    nc.scalar.activation(ec, ps_sc, AF.Exp,
                                         bias=b2_sb[0:1, 0:1], scale=SC_SCALE,
                                         accum_out=zc[:, c:c + 1])
                    eb = dramp.tile([1, CHUNK], bf16, tag="eb", name="eb")
                    nc.scalar.dma_start(eb, ec)
                    epk = epkp.tile([P, NCHUNK], bf16, tag="epk", name="epk")
                    nc.scalar.dma_start(
                        epk, eb.rearrange("o (t p) -> (o p) t", p=P))
                    epks.append(epk)
                return zc, epks

            def phase4(b, xb, zc, epks):
                """Pooling matmuls vs SBUF-kept x[s,d], divide by Z, store."""
                pp0 = pspool.tile([1, CHUNK], f32, tag="pspool", name="pp0")
                pp1 = pspool.tile([1, CHUNK], f32, tag="pspool", name="pp1")
                for c in range(NCHUNK):
                    for t in range(NG):
                        tt = c * NG + t
                        nc.tensor.matmul(pp0, epks[c][:, t:t + 1],
                                         xb[:, tt, 0:512],
                                         start=(tt == 0), stop=(tt == SUBT - 1))
                        nc.tensor.matmul(pp1, epks[c][:, t:t + 1],
                                         xb[:, tt, 512:1024],
                                         start=(tt == 0), stop=(tt == SUBT - 1))
                zt = zp.tile([1, 1], f32, tag="zt")
                nc.vector.tensor_reduce(zt, zc,
                                        axis=mybir.AxisListType.X, op=ALU.add)
                rz = zp.tile([1, 1], f32, tag="rz")
                nc.vector.reciprocal(rz, zt)
                ob = obp.tile([1, D], f32, tag="ob")
                nc.scalar.activation(ob[:, 0:512], pp0, AF.Copy,
                                     scale=rz[0:1, 0:1])
                nc.scalar.activation(ob[:, 512:1024], pp1, AF.Copy,
                                     scale=rz[0:1, 0:1])
                nc.sync.dma_start(out.ap()[b:b + 1, :], ob)

            prev = None
            for b in range(BLOC):
                xb, xtps, bcasts = phase1(b)
                if prev is not None:
                    phase4(*prev)
                xn8s = phase2(b, xtps, bcasts)
                zc, epks = phase3(b, xn8s)
                prev = (b, xb, zc, epks)
            phase4(*prev)

    nc.compile()
    return nc


_NC_CACHE = {}


def _get_nc():
    if "nc" not in _NC_CACHE:
        _NC_CACHE["nc"] = build_nc()
    return _NC_CACHE["nc"]


def _prep_host(ln_gamma, ln_beta, W1, b1, W2, b2):
    import ml_dtypes
    f8 = ml_dtypes.float8_e4m3fn
    W1p = (np.asarray(ln_gamma, np.float32)[:, None]
           * np.asarray(W1, np.float32))
    w1q = np.clip(W1p * SW, -448, 448).astype(f8)
    # interleave for DoubleRowSwInterleave: per (pair i, e-tile) the 256
    # weight bytes per partition are [A_127 B_127 ... A_0 B_0] where
    # A/B are the even/odd d-tiles of the pair and columns are reversed
    b4 = w1q.reshape(DT, P, ET, P)            # [t, p, e, m]
    A, Bm = b4[0::2], b4[1::2]                # [DP, p, e, m]
    wv = np.empty((DP, P, ET, 2 * P), f8)
    wv[..., 0::2] = A[..., ::-1]
    wv[..., 1::2] = Bm[..., ::-1]
    w1s = np.ascontiguousarray(
        wv.transpose(1, 0, 2, 3).reshape(P, DP * ET * 2 * P))
    c2 = (np.asarray(ln_beta, np.float32) @ np.asarray(W1, np.float32)
          + np.asarray(b1, np.float32))
    w2q = np.clip(
        np.ascontiguousarray(np.asarray(W2, np.float32)[:, 0]) * SW,
        -448, 448).astype(f8)
    b2s = np.asarray(b2, np.float32).reshape(1, 1)
    return w1s, np.ascontiguousarray(c2), w2q, b2s


def run_cores(inputs, trace=False, **kw):
    import ml_dtypes
    x = np.asarray(inputs["x"], np.float32)
    w1q, c2, w2q, b2s = _prep_host(inputs["ln_gamma"], inputs["ln_beta"],
                                   inputs["W1"], inputs["b1"],
                                   inputs["W2"], inputs["b2"])
    xb16 = x.astype(ml_dtypes.bfloat16)          # [B, S, D]
    xt16 = np.ascontiguousarray(xb16.transpose(0, 2, 1))  # [B, D, S]
    nc = _get_nc()
    in_maps = []
    for c in range(NCORES):
        shard = np.ascontiguousarray(
            xb16[c * BLOC:(c + 1) * BLOC].reshape(ROWS, D))
        shardT = np.ascontiguousarray(
            xt16[c * BLOC:(c + 1) * BLOC].reshape(BLOC * D, S))
        in_maps.append(dict(xbf=shard, xt=shardT, w1q=w1q, w2q=w2q,
                            c2v=c2, b2s=b2s,
                            eye=np.eye(P, dtype=ml_dtypes.bfloat16)))
    res = run_bass_kernel_spmd(nc, in_maps, core_ids=list(range(NCORES)),
                               trace=trace, **kw)
    full = np.concatenate([res.results[c]["out"] for c in range(NCORES)], axis=0)
    return full, res


def kernel(**inputs) -> np.ndarray:
    out, _ = run_cores(inputs, trace=False)
    return out.astype(np.float32)


# revision 32
# speedup vs baseline: 2.2515x; 1.0176x over previous
"""AttentionPool Trainium2 kernel v2: fp8 DoubleRow matmuls, host-side
pre-transpose, bf16 staging.

Reference computation (per batch b of 32, S=2048, D=1024):
    xn = LayerNorm(x[b])                      # over D, eps 1e-5
    h = tanh(xn @ W1 + b1)
    scores = h @ W2 + b2                      # [S]
    w = softmax(scores)
    out[b] = sum_s w[s] * x[b, s, :]

Strategy: batch axis sharded over 8 cores (4 batches each). Host stages
x twice in bf16: [s, d] layout (LN stats + pooling values) and [d, s]
layout (pre-transposed, feeds matmul1) — no on-device transposes. Host
folds ln_gamma into W1 and ln_beta@W1+b1 into c2, and scales W1/W2 by 64
so fp8e4 (e4m3) quantization stays in the normal range; the inverse
scales ride the ACT activation `scale` operand.

Per core, per batch:
  - LN stats on DVE (bn_stats/bn_aggr + Newton rsqrt) in [s,d] layout;
    mu and rstd*16 bounce through DRAM and are broadcast-loaded as
    [128, S] tiles (per-free-column vectors for the transposed layout).
  - T-space normalize on DVE: xn8 = (xT - mu_b) * rs_b  -> fp8e4,
    written as [128, 2, S] d-pair tiles (DoubleRow operand layout).
  - matmul1: fp8 DoubleRow (K=256 per instruction), PSUM accumulate,
    tanh+c2 on ACT -> fp8 h pair tiles; scores via fp8 DoubleRow,
    exp on ACT (accum_out gives Z per chunk).
  - pooling via bf16 matmuls against the [s,d] x staging tiles kept in
    SBUF; divide by Z at the end.
Engine queues: GpSimd = x[s,d] loads; Sync = xT loads + stat stores +
output; Scalar(ACT) = broadcast loads + e-scatter bounces.
"""
import sys
import os

sys.path.insert(0, '/opt/trn_rl_repo')

import numpy as np

import concourse.bass as bass
import concourse.tile as tile
from concourse import bacc, mybir
from concourse.bass_utils import run_bass_kernel_spmd

P = 128
D = 1024
S = 2048
B = 32
NCORES = 8
BLOC = B // NCORES            # batches per core
ROWS = BLOC * S               # 8192 rows per core
DT = D // P                   # 8 d-tiles
ET = D // P                   # 8 e-tiles
DP = DT // 2                  # 4 d-pairs (DoubleRow)
EP = ET // 2                  # 4 e-pairs
SUBT = S // P                 # 16 subtiles per batch
NG = 4                        # subtiles per stats group
CHUNK = 512                   # matmul moving free dim
NCHUNK = S // CHUNK           # 4 chunks per batch

SW = 64.0                     # W1/W2 fp8 pre-scale (host)
SX = 16.0                     # xn fp8 pre-scale (device)
MM1_SCALE = 1.0 / (SW * SX)   # applied in tanh activation
SC_SCALE = 1.0 / SW           # applied in exp activation

f32 = mybir.dt.float32
bf16 = mybir.dt.bfloat16
fp8 = mybir.dt.float8e4
AF = mybir.ActivationFunctionType
ALU = mybir.AluOpType
DR = mybir.MatmulPerfMode.DoubleRow
DRSWI = mybir.MatmulPerfMode.DoubleRowSwInterleave


def build_nc():
    nc = bacc.Bacc("TRN2", target_bir_lowering=False, num_devices=NCORES)

    xbf = nc.dram_tensor("xbf", [ROWS, D], bf16, kind="ExternalInput")
    xt = nc.dram_tensor("xt", [BLOC * D, S], bf16, kind="ExternalInput")
    # W1 pre-interleaved on host for DoubleRowSwInterleave ldweights:
    # per partition d_p, free dim = [A_127 B_127 ... A_0 B_0] per (pair, e)
    w1q = nc.dram_tensor("w1q", [P, DP * ET * 2 * P], fp8,
                         kind="ExternalInput")
    w2q = nc.dram_tensor("w2q", [D], fp8, kind="ExternalInput")
    c2v = nc.dram_tensor("c2v", [D], f32, kind="ExternalInput")
    b2s = nc.dram_tensor("b2s", [1, 1], f32, kind="ExternalInput")
    eye = nc.dram_tensor("eye", [P, P], bf16, kind="ExternalInput")
    out = nc.dram_tensor("out", [BLOC, D], f32, kind="ExternalOutput")

    with tile.TileContext(nc) as tc:
        with (
            tc.tile_pool(name="consts", bufs=1) as consts,
            tc.tile_pool(name="xb", bufs=2) as xbp,            # [128,16,1024] bf16
            tc.tile_pool(name="stats", bufs=8) as statp,
            tc.tile_pool(name="bcast", bufs=8) as bcp,         # [128,512] bf16
            tc.tile_pool(name="xtp", bufs=8) as xtpp,          # [128,2,1024] bf16
            tc.tile_pool(name="xn8", bufs=16) as xn8p,         # [128,2,1024] fp8
            tc.tile_pool(name="h8", bufs=8) as h8p,            # [128,2,512] fp8
            tc.tile_pool(name="ec", bufs=4) as ecp,            # [1,512] bf16
            tc.tile_pool(name="epk", bufs=8) as epkp,          # [128,4] bf16
            tc.tile_pool(name="z", bufs=4) as zp,              # tiny scalars
            tc.tile_pool(name="ob", bufs=2) as obp,
            tc.tile_pool(name="psmm", bufs=4, space="PSUM") as psmm,
            tc.tile_pool(name="pssc", bufs=1, space="PSUM") as pssc,
            tc.tile_pool(name="pspool", bufs=2, space="PSUM") as pspool,
            tc.tile_pool(name="pst", bufs=1, space="PSUM") as pstp,
            tc.tile_pool(name="dram", bufs=8, space="DRAM") as dramp,
        ):
            # ---- constants ----
            w1_sb = consts.tile([P, DP, ET, 2 * P], fp8)   # interleaved pairs
            nc.scalar.dma_start(
                w1_sb, w1q.ap().rearrange("p (i e m) -> p i e m", i=DP, e=ET))
            # dual-fp8 ldweights needs a 16B-aligned outer free step: pad
            # each e-tile's single weight column out to 16 bytes
            w2_sb = consts.tile([P, ET, 16], fp8)
            nc.scalar.dma_start(
                w2_sb[:, :, 0:1],
                w2q.ap().rearrange("(t p) -> p t", p=P).unsqueeze(2))
            c2_sb = consts.tile([P, ET], f32)
            nc.scalar.dma_start(c2_sb, c2v.ap().rearrange("(t p) -> p t", p=P))
            b2_sb = consts.tile([1, 1], f32)
            nc.sync.dma_start(b2_sb, b2s.ap())
            eye_sb = consts.tile([P, P], bf16)
            nc.sync.dma_start(eye_sb, eye.ap())

            xbf3 = xbf.ap().rearrange("(b t p) d -> b t p d", b=BLOC, p=P)
            xt4 = xt.ap().rearrange("(b u p) s -> b p u s", b=BLOC, p=P)

            HB = S // 2           # half-batch token granularity
            HSUB = SUBT // 2      # 8 subtiles per half

            def phase1(b):
                """Load x[s,d] and xT; LN stats + quake+1-Newton rsqrt + DRAM
                bounce per half-batch (1024 tokens) so the first half's
                broadcast round-trip hides under the second half's stats.
                xT tiles are per-half so chunk-0 matmuls only wait on 2MB of
                loads; stat stores ride gpsimd to keep sync free for xT."""
                xb = xbp.tile([P, SUBT, D], bf16, tag="xb")
                xtps = [[None] * DP, [None] * DP]
                for h in range(2):
                    for i in range(DP):
                        xtp = xtpp.tile([P, 2, HB], bf16, tag="xtp",
                                        name="xtp")
                        nc.sync.dma_start(
                            xtp, xt4[b, :, 2 * i:2 * i + 2,
                                     h * HB:(h + 1) * HB])
                        xtps[h][i] = xtp
                statd = dramp.tile([2, S], bf16, tag="statd", name="statd")
                bcasts = []
                for h in range(2):
                    t0 = h * HSUB
                    hs = slice(h * HB, (h + 1) * HB)
                    mvh = statp.tile([P, HSUB, 2], f32, tag="mvh")
                    for g in range(2):
                        tg = t0 + g * NG
                        nc.gpsimd.dma_start(
                            xb[:, tg:tg + NG, :],
                            xbf3[b, tg:tg + NG].rearrange("t p d -> p t d"))
                        for s in range(NG):
                            st = statp.tile([P, 2, 6], f32, tag="bnst")
                            nc.vector.bn_stats(st[:, 0, :],
                                               xb[:, tg + s, 0:512])
                            nc.vector.bn_stats(st[:, 1, :],
                                               xb[:, tg + s, 512:1024])
                            nc.vector.bn_aggr(mvh[:, g * NG + s, :], st)
                    # rstd = rsqrt(var+eps): quake seed + 1 Newton step
                    var = statp.tile([P, HSUB], f32, tag="var")
                    nc.vector.tensor_scalar(out=var, in0=mvh[:, :, 1],
                                            scalar1=1e-5, scalar2=0.5,
                                            op0=ALU.add, op1=ALU.mult)
                    y = statp.tile([P, HSUB], f32, tag="y")
                    yi = y.bitcast(mybir.dt.int32)
                    vi = var.bitcast(mybir.dt.int32)
                    nc.vector.tensor_scalar(out=yi, in0=vi, scalar1=0x800000,
                                            scalar2=None, op0=ALU.add)
                    nc.vector.tensor_scalar(out=yi, in0=yi, scalar1=1,
                                            scalar2=None,
                                            op0=ALU.logical_shift_right)
                    nc.vector.tensor_scalar(out=yi, in0=yi, scalar1=-1,
                                            scalar2=0x5f3759df,
                                            op0=ALU.mult, op1=ALU.add)
                    tny = statp.tile([P, HSUB], f32, tag="tny")
                    nc.vector.tensor_tensor(tny, y, y, ALU.mult)
                    nc.vector.tensor_tensor(tny, tny, var, ALU.mult)
                    nc.vector.tensor_scalar(out=tny, in0=tny, scalar1=-1.0,
                                            scalar2=1.5,
                                            op0=ALU.mult, op1=ALU.add)
                    nc.vector.tensor_tensor(y, y, tny, ALU.mult)
                    # pack mu | rstd*SX (bf16), PE transpose [128,16]->[16,128]
                    # so the DRAM bounce store is contiguous
                    mr = statp.tile([P, 2 * HSUB], bf16, tag="mr")
                    nc.vector.tensor_copy(mr[:, 0:HSUB], mvh[:, :, 0])
                    nc.vector.tensor_scalar(out=mr[:, HSUB:2 * HSUB], in0=y,
                                            scalar1=SX, scalar2=None,
                                            op0=ALU.mult)
                    mrt = pstp.tile([2 * HSUB, P], bf16, tag="mrt")
                    nc.tensor.transpose(mrt, mr, eye_sb)
                    mrs = statp.tile([2 * HSUB, P], bf16, tag="mrs")
                    nc.scalar.activation(mrs, mrt, AF.Copy)
                    nc.gpsimd.dma_start(statd[0:2, hs], mrs)
                    mu_bh = bcp.tile([P, HB], bf16, tag="mu_b", name="mu_b")
                    nc.scalar.dma_start(
                        mu_bh, statd[0:1, hs].to_broadcast((P, HB)))
                    rs_bh = bcp.tile([P, HB], bf16, tag="rs_b", name="rs_b")
                    nc.scalar.dma_start(
                        rs_bh, statd[1:2, hs].to_broadcast((P, HB)))
                    bcasts.append((mu_bh, rs_bh))
                return xb, xtps, bcasts

            def phase2(b, xtps, bcasts):
                """T-space normalize to fp8 pair tiles, half by half."""
                xn8s = [[None] * DP, [None] * DP]
                for h in range(2):
                    mu_bh, rs_bh = bcasts[h]
                    for i in range(DP):
                        xn8 = xn8p.tile([P, 2, HB], fp8, tag="xn8",
                                        name="xn8")
                        for j in range(2):
                            nc.vector.tensor_tensor(xtps[h][i][:, j],
                                                    xtps[h][i][:, j], mu_bh,
                                                    ALU.subtract)
                            nc.vector.tensor_tensor(xn8[:, j],
                                                    xtps[h][i][:, j], rs_bh,
                                                    ALU.mult)
                        xn8s[h][i] = xn8
                return xn8s

            def phase3(b, xn8s):
                """fp8 DoubleRow matmul1 + tanh + scores + exp per chunk."""
                zc = zp.tile([1, NCHUNK], f32, tag="zc", name="zc")
                epks = []
                for c in range(NCHUNK):
                    ch = c // 2
                    cs = slice((c % 2) * CHUNK, (c % 2 + 1) * CHUNK)
                    h8s = [h8p.tile([P, 2, CHUNK], fp8, tag="h8", name="h8")
                           for _ in range(EP)]
                    for e in range(ET):
                        ps = psmm.tile([P, CHUNK], f32, tag="psmm")
                        for i in range(DP):
                            nc.tensor.matmul(
                                ps, w1_sb[:, i, e, :],
                                xn8s[ch][i][:, :, cs],
                                start=(i == 0), stop=(i == DP - 1),
                                perf_mode=DRSWI)
                        nc.scalar.activation(h8s[e // 2][:, e % 2, :], ps,
                                             AF.Tanh, bias=c2_sb[:, e:e + 1],
                                             scale=MM1_SCALE)
                    ps_sc = pssc.tile([1, CHUNK], f32, tag="pssc")
                    for k in range(EP):
                        nc.tensor.matmul(ps_sc,
                                         w2_sb[:, 2 * k:2 * k + 2, 0:1],
                                         h8s[k], start=(k == 0),
                                         stop=(k == EP - 1), perf_mode=DR)
                    ec = ecp.tile([1, CHUNK], bf16, tag="ec", name="ec")
                    nc.scalar.activation(ec, ps_sc, AF.Exp,
                                         bias=b2_sb[0:1, 0:1], scale=SC_SCALE,
                                         accum_out=zc[:, c:c + 1])
                    eb = dramp.tile([1, CHUNK], bf16, tag="eb", name="eb")
                    nc.scalar.dma_start(eb, ec)
                    epk = epkp.tile([P, NCHUNK], bf16, tag="epk", name="epk")
                    nc.scalar.dma_start(
                        epk, eb.rearrange("o (t p) -> (o p) t", p=P))
                    epks.append(epk)
                return zc, epks

            def phase4(b, xb, zc, epks):
                """Pooling matmuls vs SBUF-kept x[s,d], divide by Z, store."""
                pp0 = pspool.tile([1, CHUNK], f32, tag="pspool", name="pp0")
                pp1 = pspool.tile([1, CHUNK], f32, tag="pspool", name="pp1")
                for c in range(NCHUNK):
                    for t in range(NG):
                        tt = c * NG + t
                        nc.tensor.matmul(pp0, epks[c][:, t:t + 1],
                                         xb[:, tt, 0:512],
                                         start=(tt == 0), stop=(tt == SUBT - 1))
                        nc.tensor.matmul(pp1, epks[c][:, t:t + 1],
                                         xb[:, tt, 512:1024],
                                         start=(tt == 0), stop=(tt == SUBT - 1))
                zt = zp.tile([1, 1], f32, tag="zt")
                nc.vector.tensor_reduce(zt, zc,
                                        axis=mybir.AxisListType.X, op=ALU.add)
                rz = zp.tile([1, 1], f32, tag="rz")
                nc.vector.reciprocal(rz, zt)
                ob = obp.tile([1, D], f32, tag="ob")
                nc.scalar.activation(ob[:, 0:512], pp0, AF.Copy,
                                     scale=rz[0:1, 0:1])
                nc.scalar.activation(ob[:, 512:1024], pp1, AF.Copy,
                                     scale=rz[0:1, 0:1])
                nc.sync.dma_start(out.ap()[b:b + 1, :], ob)

            prev = None
            for b in range(BLOC):
                xb, xtps, bcasts = phase1(b)
                if prev is not None:
                    phase4(*prev)
                xn8s = phase2(b, xtps, bcasts)
                zc, epks = phase3(b, xn8s)
                prev = (b, xb, zc, epks)
            phase4(*prev)

    nc.compile()
    return nc


_NC_CACHE = {}


def _get_nc():
    if "nc" not in _NC_CACHE:
        _NC_CACHE["nc"] = build_nc()
    return _NC_CACHE["nc"]


def _prep_host(ln_gamma, ln_beta, W1, b1, W2, b2):
    import ml_dtypes
    f8 = ml_dtypes.float8_e4m3fn
    W1p = (np.asarray(ln_gamma, np.float32)[:, None]
           * np.asarray(W1, np.float32))
    w1q = np.clip(W1p * SW, -448, 448).astype(f8)
    # interleave for DoubleRowSwInterleave: per (pair i, e-tile) the 256
    # weight bytes per partition are [A_127 B_127 ... A_0 B_0] where
    # A/B are the even/odd d-tiles of the pair and columns are reversed
    b4 = w1q.reshape(DT, P, ET, P)            # [t, p, e, m]
    A, Bm = b4[0::2], b4[1::2]                # [DP, p, e, m]
    wv = np.empty((DP, P, ET, 2 * P), f8)
    wv[..., 0::2] = A[..., ::-1]
    wv[..., 1::2] = Bm[..., ::-1]
    w1s = np.ascontiguousarray(
        wv.transpose(1, 0, 2, 3).reshape(P, DP * ET * 2 * P))
    c2 = (np.asarray(ln_beta, np.float32) @ np.asarray(W1, np.float32)
          + np.asarray(b1, np.float32))
    w2q = np.clip(
        np.ascontiguousarray(np.asarray(W2, np.float32)[:, 0]) * SW,
        -448, 448).astype(f8)
    b2s = np.asarray(b2, np.float32).reshape(1, 1)
    return w1s, np.ascontiguousarray(c2), w2q, b2s


def run_cores(inputs, trace=False, **kw):
    import ml_dtypes
    x = np.asarray(inputs["x"], np.float32)
    w1q, c2, w2q, b2s = _prep_host(inputs["ln_gamma"], inputs["ln_beta"],
                                   inputs["W1"], inputs["b1"],
                                   inputs["W2"], inputs["b2"])
    xb16 = x.astype(ml_dtypes.bfloat16)          # [B, S, D]
    xt16 = np.ascontiguousarray(xb16.transpose(0, 2, 1))  # [B, D, S]
    nc = _get_nc()
    in_maps = []
    for c in range(NCORES):
        shard = np.ascontiguousarray(
            xb16[c * BLOC:(c + 1) * BLOC].reshape(ROWS, D))
        shardT = np.ascontiguousarray(
            xt16[c * BLOC:(c + 1) * BLOC].reshape(BLOC * D, S))
        in_maps.append(dict(xbf=shard, xt=shardT, w1q=w1q, w2q=w2q,
                            c2v=c2, b2s=b2s,
                            eye=np.eye(P, dtype=ml_dtypes.bfloat16)))
    res = run_bass_kernel_spmd(nc, in_maps, core_ids=list(range(NCORES)),
                               trace=trace, **kw)
    full = np.concatenate([res.results[c]["out"] for c in range(NCORES)], axis=0)
    return full, res


def kernel(**inputs) -> np.ndarray:
    out, _ = run_cores(inputs, trace=False)
    return out.astype(np.float32)
